# revision 8
# baseline (speedup 1.0000x reference)
"""Trainium2 Bass kernel for nn_DotAttentionX (rank-1 score attention).

Math (per (b,h) head, derived from the reference):
    c[l] = sum_d q[l,d]            (qsum)
    k[s] = sum_d key[s,d]          (ksum)
    score[l,s] = c[l]*k[s]
    attn = softmax(score, axis=s)
    out[l,d] = sum_s attn[l,s] v[s,d]

Row max of score has closed form m[l] = max(c[l]*kmax, c[l]*kmin), so the
whole [L,S] max reduction is never materialized.

Device mapping (8 cores, B*H = 16 heads -> 2 heads/core, no comms):
  Per head, in [s, l] layout (s on partitions so the PE can contract over s):
    E[s,l] = k[s]*c[l] - m[l]   -- rank-2 outer product, computed by the PE
             as a K=6 fp32r matmul of exact 12-bit hi/lo splits (fp32r is
             fp32 truncated to 12 explicit mantissa bits; splitting k, c and
             -m into two 12-bit pieces makes every product exact, so E is
             fp32-faithful at full PE speed; plain fp32 matmul is 4x slower).
    W[s,l] = exp(E)             -- ScalarE activation, PSUM -> SBUF fp32r
    outT[d,l] += V[s,d]^T @ W   -- PE, V stationary with a ones column
                                   appended so row 64 accumulates Z = sum_s W
    out[l,d] = outT[d,l] / Z[l] -- PE transpose + DVE reciprocal + multiply
  fp32r truncation bias (-2^-13 average) is compensated by scaling V by
  (1+2^-13) and adding ln(1+2^-13) to -m before the exp.
"""

import sys

sys.path.insert(0, "/opt/trn_rl_repo")

import numpy as np

import concourse.bass as bass
import concourse.mybir as mybir
import concourse.tile as tile
from concourse import bacc, bass_isa
from concourse.bass_utils import run_bass_kernel_spmd
from concourse.masks import make_identity

B, H, L, S, D = 2, 8, 2048, 2048, 64
N_CORES = 8
HEADS_PER_CORE = (B * H) // N_CORES  # 2
P = 128
NCH = L // P  # 16 chunks of 128
HALF = 1024  # l-half size
F32 = mybir.dt.float32
F32R = mybir.dt.float32r
U32 = mybir.dt.uint32

TRUNC_MASK = 0xFFFFF000  # keep 11 explicit mantissa bits (fp32r = e8m11 RNE)
COMP_MUL = 1.0  # fp32r rounds RNE (unbiased) -> no compensation
COMP_ADD = 0.0

_CACHED = {}


def _build_program(reps=1):
    nc = bacc.Bacc("TRN2", target_bir_lowering=False, debug=False,
                   num_devices=N_CORES)
    G = HEADS_PER_CORE
    qd = nc.dram_tensor("q", [G, L, D], F32, kind="ExternalInput").ap()
    kd = nc.dram_tensor("k", [G, S, D], F32, kind="ExternalInput").ap()
    vd = nc.dram_tensor("v", [G, S, D], F32, kind="ExternalInput").ap()
    od = nc.dram_tensor("o", [G, L, D], F32, kind="ExternalOutput").ap()

    import contextlib

    def rep_ctx(tc):
        return tc.For_i(0, reps, 1, hint_engines=(
            mybir.EngineType.PE, mybir.EngineType.Activation,
            mybir.EngineType.DVE, mybir.EngineType.SP,
            mybir.EngineType.Pool)) if reps > 1 else contextlib.nullcontext()

    with tile.TileContext(nc) as tc:
        with (
            tc.tile_pool(name="const", bufs=1) as const,
            tc.tile_pool(name="loads", bufs=2) as loads,
            tc.tile_pool(name="vt", bufs=2) as vtp,
            tc.tile_pool(name="stats", bufs=2) as stats,
            tc.tile_pool(name="rows", bufs=2) as rows,
            tc.tile_pool(name="wp", bufs=4) as wp,
            tc.tile_pool(name="osb", bufs=2) as osb,
            tc.tile_pool(name="ost", bufs=2) as ostp,
            tc.tile_pool(name="rz", bufs=4) as rzp,
            tc.tile_pool(name="pe", bufs=2, space="PSUM") as pe,
            tc.tile_pool(name="po", bufs=1, space="PSUM") as po,
            tc.tile_pool(name="pt", bufs=2, space="PSUM") as pt,
        ):
            idn = const.tile([P, P], F32)
            make_identity(nc, idn[:])

            rc = rep_ctx(tc)
            rc.__enter__()
            for g in range(G):
                # ---- loads: [L, D] -> [128, 16, D] (partition = l%... l = c*128+p)
                q3 = loads.tile([P, NCH, D], F32, tag="q3")
                k3 = loads.tile([P, NCH, D], F32, tag="k3")
                v3 = loads.tile([P, NCH, D], F32, tag="v3")
                nc.sync.dma_start(out=q3[:], in_=qd[g].rearrange("(c p) d -> p c d", p=P))
                nc.sync.dma_start(out=k3[:], in_=kd[g].rearrange("(c p) d -> p c d", p=P))
                nc.sync.dma_start(out=v3[:], in_=vd[g].rearrange("(c p) d -> p c d", p=P))

                # V with compensation, cast to fp32r, ones column appended
                vt = vtp.tile([P, NCH, D + 1], F32R, tag="vt")
                nc.vector.tensor_scalar_mul(vt[:, :, 0:D], v3[:], COMP_MUL)
                ones16 = stats.tile([P, NCH], F32, tag="ones16")
                nc.vector.memset(ones16[:], 1.0)
                nc.vector.tensor_copy(vt[:, :, D], ones16[:])

                # ---- row sums
                qs = stats.tile([P, NCH], F32, tag="qs")
                ks = stats.tile([P, NCH], F32, tag="ks")
                nc.vector.reduce_sum(out=qs[:], in_=q3[:], axis=mybir.AxisListType.X)
                nc.vector.reduce_sum(out=ks[:], in_=k3[:], axis=mybir.AxisListType.X)

                # ---- global kmax / kmin (via gpsimd cross-partition max)
                kx = stats.tile([P, 1], F32, tag="kx")
                nks = stats.tile([P, NCH], F32, tag="nks")
                nkx = stats.tile([P, 1], F32, tag="nkx")
                nc.vector.reduce_max(out=kx[:], in_=ks[:], axis=mybir.AxisListType.X)
                nc.vector.tensor_scalar_mul(nks[:], ks[:], -1.0)
                nc.vector.reduce_max(out=nkx[:], in_=nks[:], axis=mybir.AxisListType.X)
                kxa = stats.tile([P, 1], F32, tag="kxa")    # kmax in every partition
                nkna = stats.tile([P, 1], F32, tag="nkna")  # -kmin in every partition
                nc.gpsimd.partition_all_reduce(kxa[:], kx[:], P, bass_isa.ReduceOp.max)
                nc.gpsimd.partition_all_reduce(nkna[:], nkx[:], P, bass_isa.ReduceOp.max)
                nkmax = stats.tile([P, 1], F32, tag="nkmax")  # -kmax
                nc.vector.tensor_scalar_mul(nkmax[:], kxa[:], -1.0)

                # ---- negm = min(c*(-kmax), c*(-kmin)) + COMP_ADD, then 12-bit splits
                # TQ columns: [c1 | c2 | nm1 | nm2], TK columns: [k1 | k2]
                tq = stats.tile([P, 4 * NCH], F32, tag="tq")
                tk = stats.tile([P, 2 * NCH], F32, tag="tk")
                t1 = stats.tile([P, NCH], F32, tag="t1")
                t2 = stats.tile([P, NCH], F32, tag="t2")
                nc.vector.tensor_scalar_mul(t1[:], qs[:], nkmax[:])
                nc.vector.tensor_scalar_mul(t2[:], qs[:], nkna[:])
                negm = stats.tile([P, NCH], F32, tag="negm")
                nc.vector.tensor_tensor(negm[:], t1[:], t2[:], mybir.AluOpType.min)
                nc.vector.tensor_scalar_add(negm[:], negm[:], COMP_ADD)

                nc.vector.tensor_scalar(
                    tq[:, 0:NCH].bitcast(U32), qs[:].bitcast(U32),
                    TRUNC_MASK, None, op0=mybir.AluOpType.bitwise_and)
                nc.vector.tensor_sub(tq[:, NCH : 2 * NCH], qs[:], tq[:, 0:NCH])
                nc.vector.tensor_scalar(
                    tq[:, 2 * NCH : 3 * NCH].bitcast(U32), negm[:].bitcast(U32),
                    TRUNC_MASK, None, op0=mybir.AluOpType.bitwise_and)
                nc.vector.tensor_sub(tq[:, 3 * NCH : 4 * NCH], negm[:], tq[:, 2 * NCH : 3 * NCH])
                nc.vector.tensor_scalar(
                    tk[:, 0:NCH].bitcast(U32), ks[:].bitcast(U32),
                    TRUNC_MASK, None, op0=mybir.AluOpType.bitwise_and)
                nc.vector.tensor_sub(tk[:, NCH : 2 * NCH], ks[:], tk[:, 0:NCH])

                # ---- transpose the split planes: [128, 4*16] -> [64, 128] etc.
                tqp = pt.tile([4 * NCH, P], F32, tag="pt")
                nc.tensor.transpose(tqp[:], tq[:], idn[:])
                tqr = rows.tile([4 * NCH, P], F32R, tag="tqr")
                nc.vector.tensor_copy(tqr[:], tqp[:])
                tkp = pt.tile([2 * NCH, P], F32, tag="pt")
                nc.tensor.transpose(tkp[:], tk[:], idn[:])
                tkr = rows.tile([2 * NCH, P], F32R, tag="tkr")
                nc.vector.tensor_copy(tkr[:], tkp[:])

                # ---- flatten to contraction-row layout
                # QROWS partitions: (c1, c2, c1, c2, nm1, nm2)
                # KROWS partitions: (k1, k1, k2, k2, 1, 1)
                qrows = rows.tile([6, S], F32R, tag="qrows")
                krows = rows.tile([6, S], F32R, tag="krows")
                for p_dst, lo in [(0, 0), (1, NCH), (2, 0), (3, NCH), (4, 2 * NCH), (5, 3 * NCH)]:
                    nc.sync.dma_start(out=qrows[p_dst : p_dst + 1, :],
                                      in_=tqr[lo : lo + NCH, :])
                # rows 4-5 stay ones (memset doesn't take fp32r; write the bits)
                nc.vector.memset(krows[:].bitcast(U32), 0x3F800000)
                for p_dst, lo in [(0, 0), (1, 0), (2, NCH), (3, NCH)]:
                    nc.sync.dma_start(out=krows[p_dst : p_dst + 1, :],
                                      in_=tkr[lo : lo + NCH, :])

                # ---- main loop over l-halves
                for h in range(2):
                    outt = po.tile([D + 1, HALF], F32, tag="po")
                    for j in range(NCH):
                        e = pe.tile([P, HALF], F32, tag="pe")
                        for half2 in range(2):
                            nc.tensor.matmul(
                                e[:, half2 * 512 : (half2 + 1) * 512],
                                krows[:, j * P : (j + 1) * P],
                                qrows[:, h * HALF + half2 * 512 : h * HALF + (half2 + 1) * 512],
                                start=True, stop=True)
                        w = wp.tile([P, HALF], F32R, tag="w")
                        nc.scalar.activation(w[:], e[:], mybir.ActivationFunctionType.Exp)
                        for half2 in range(2):
                            nc.tensor.matmul(
                                outt[:, half2 * 512 : (half2 + 1) * 512],
                                vt[:, j, :],
                                w[:, half2 * 512 : (half2 + 1) * 512],
                                start=(j == 0), stop=(j == NCH - 1))

                    # ---- epilogue: transpose to [l, d], divide by Z, store
                    ot = osb.tile([D + 1, HALF], F32, tag="osb")
                    nc.vector.tensor_copy(ot[:], outt[:])
                    ost = ostp.tile([P, HALF // P, D], F32, tag="ost")
                    for c in range(HALF // P):
                        tp = pt.tile([P, D + 1], F32, tag="pt")
                        nc.tensor.transpose(tp[:], ot[:, c * P : (c + 1) * P],
                                            idn[0 : D + 1, 0 : D + 1])
                        rz = rzp.tile([P, 1], F32, tag="rz")
                        nc.vector.reciprocal(rz[:], tp[:, D : D + 1])
                        nc.vector.tensor_scalar_mul(ost[:, c, :], tp[:, 0:D], rz[:])
                    nc.sync.dma_start(
                        out=od[g, h * HALF : (h + 1) * HALF, :].rearrange(
                            "(c p) d -> p c d", p=P),
                        in_=ost[:])
            rc.__exit__(None, None, None)

    nc.compile()
    return nc


def get_program(reps=1):
    if reps not in _CACHED:
        _CACHED[reps] = _build_program(reps)
    return _CACHED[reps]


def make_in_maps(query, key, value):
    q = np.ascontiguousarray(query.reshape(B * H, L, D), dtype=np.float32)
    k = np.ascontiguousarray(key.reshape(B * H, S, D), dtype=np.float32)
    v = np.ascontiguousarray(value.reshape(B * H, S, D), dtype=np.float32)
    maps = []
    for i in range(N_CORES):
        sl = slice(i * HEADS_PER_CORE, (i + 1) * HEADS_PER_CORE)
        maps.append({"q": q[sl], "k": k[sl], "v": v[sl]})
    return maps


def kernel(query, key, value):
    nc = get_program(1)
    res = run_bass_kernel_spmd(nc, make_in_maps(query, key, value),
                               list(range(N_CORES)))
    out = np.concatenate([res.results[i]["o"] for i in range(N_CORES)], axis=0)
    return out.reshape(B, H, L, D).astype(np.float32)
